# revision 16
# baseline (speedup 1.0000x reference)
"""Multi-head attention (B=2, S=2048, D=1024, H=16) on 8 Trainium2 cores.

Sharding: tensor-parallel over heads (2 heads/core) for QKV+attention, then a
per-batch AllToAll reshards so each core out-projects its own 256-row slice of
the sequence.  Host concatenates the per-core slices.

Engine plan:
  - Row-tiled concurrent scores: head 0 lives on PE rows 0:64, head 1 on rows
    64:128; their K^T.T@Q^T matmuls use disjoint PE row-groups and issue
    back-to-back (~4ns apart), near-doubling bf16 scores throughput.
  - exp split: head-0 scores on the Scalar engine (true EXP), head-1 scores as
    a one-op DVE Schraudolph (int16 bits = A*s + B, bitcast to bf16).
  - Software-pipelined attention: AV matmuls of chunk c-2 issue between the
    score matmuls of chunks c and c+1 so the PE never waits on exp.
  - All PSUM->SBUF eviction copies in the prologue run on the Scalar engine;
    the DVE queue carries only Schraudolph + normalize work.
  - Per-unit softmax normalization is deferred into the next unit's chunk
    stream; the AllToAll windows are filled with out-projection work.
"""
import sys

sys.path.insert(0, "/opt/trn_rl_repo")

import numpy as np
import ml_dtypes

import concourse.bass as bass
import concourse.tile as tile
from concourse import bacc, mybir
from concourse import bass_utils
from concourse.alu_op_type import AluOpType

B = 2
S = 2048
D = 1024
H = 16
DH = 64
N_CORES = 8
HEADS_PER_CORE = H // N_CORES          # 2
S_SLICE = S // N_CORES                 # 256
N_CH = D // 128                        # 8 contraction chunks
N_KC = S // 128                        # 16 key chunks
N_QT = S // 512                        # 4 query tiles

F32 = mybir.dt.float32
BF16 = mybir.dt.bfloat16
I16 = mybir.dt.int16

LOG2E = 1.4426950408889634
SCALE = DH ** -0.5
C_ADJ = 0.04303
# bf16 Schraudolph: i16 = 128*((s*SCALE)*log2e + 127 - c), bitcast to bf16
SCH_A = SCALE * 128.0 * LOG2E
SCH_B = 128.0 * (127.0 - C_ADJ)

EXP = mybir.ActivationFunctionType.Exp

_compiled = None
last_results = None


def _build():
    nc = bacc.Bacc(
        "TRN2",
        target_bir_lowering=False,
        debug=False,
        enable_asserts=True,
        num_devices=N_CORES,
    )

    xtb = nc.dram_tensor("xtb", [B, 128, N_CH, S], BF16, kind="ExternalInput").ap()
    wqt = nc.dram_tensor("wqt", [128, N_CH, 128], BF16, kind="ExternalInput").ap()
    wkt = nc.dram_tensor("wkt", [128, N_CH, 128], BF16, kind="ExternalInput").ap()
    wvt = nc.dram_tensor("wvt", [128, N_CH, 128], BF16, kind="ExternalInput").ap()
    wot = nc.dram_tensor("wot", [128, N_CH, D], BF16, kind="ExternalInput").ap()
    bb = nc.dram_tensor("bb", [128, D], F32, kind="ExternalInput").ap()
    oc = nc.dram_tensor("oc", [B, S_SLICE, D], F32, kind="ExternalOutput").ap()

    with tile.TileContext(nc) as tc:
        with (
            tc.tile_pool(name="w", bufs=1) as wp,
            tc.tile_pool(name="qkt", bufs=1) as qktp,
            tc.tile_pool(name="vsb", bufs=1) as vsbp,
            tc.tile_pool(name="xsb", bufs=1) as xsbp,
            tc.tile_pool(name="pt", bufs=3) as ptp,
            tc.tile_pool(name="avc", bufs=2) as avcp,
            tc.tile_pool(name="norm", bufs=2) as normp,
            tc.tile_pool(name="x2", bufs=1) as x2p,
            tc.tile_pool(name="outsb", bufs=2) as outp,
            tc.tile_pool(name="dram", bufs=1, space="DRAM") as dram,
            tc.tile_pool(name="dramsc", bufs=4, space="DRAM") as dramsc,
            tc.tile_pool(name="sps", bufs=2, space="PSUM") as sps,
            tc.tile_pool(name="avps", bufs=1, space="PSUM") as avps,
            tc.tile_pool(name="ops", bufs=2, space="PSUM") as ops,
        ):
            # ---- weights ----
            wq_sb = wp.tile([128, N_CH, 128], BF16)
            nc.sync.dma_start(wq_sb[:], wqt[:])
            wk_sb = wp.tile([128, N_CH, 128], BF16)
            nc.sync.dma_start(wk_sb[:], wkt[:])
            wv_sb = wp.tile([128, N_CH, 128], BF16)
            nc.sync.dma_start(wv_sb[:], wvt[:])

            # ---- starting gun: barrier+AllGather absorbs launch skew ----
            gun_in = dram.tile([1, 16], F32, name="gun_in")
            gun_out = dram.tile([N_CORES, 16], F32, name="gun_out")
            gun_sb = wp.tile([1, 16], F32)
            nc.gpsimd.memset(gun_sb[:], 0.0)
            nc.sync.dma_start(gun_in[:], gun_sb[:])
            nc.gpsimd.collective_compute(
                "AllGather", mybir.AluOpType.bypass,
                replica_groups=[list(range(N_CORES))],
                ins=[gun_in[:]], outs=[gun_out[:]],
            )

            # ---- x loads: batch 0 chunk-major, batch 1 tile-major ----
            x_sb = [xsbp.tile([128, N_CH, S], BF16, tag=f"x{b}", name=f"x{b}")
                    for b in range(B)]
            for ch in range(N_CH):
                nc.sync.dma_start(x_sb[0][:, ch, :], xtb[0, :, ch, :])

            # ---- PE warmup (HAM clock ramp) while x streams in ----
            warm = wp.tile([128, 512], BF16)
            nc.gpsimd.memset(warm[:], 0.0)
            for _ in range(16):
                wps = ops.tile([128, 512], F32, tag="o", name="wps")
                nc.tensor.matmul(wps[:], lhsT=warm[:, 0:128], rhs=warm[:],
                                 start=True, stop=True)

            Qt = [qktp.tile([128, S], BF16, tag=f"qt{b}", name=f"qt{b}")
                  for b in range(B)]
            Kt = [qktp.tile([128, S], BF16, tag=f"kt{b}", name=f"kt{b}")
                  for b in range(B)]
            Vt = [vsbp.tile([128, N_KC, 2, 80], BF16, tag=f"v{b}",
                            name=f"v{b}") for b in range(B)]
            for b in range(B):
                nc.gpsimd.memset(Vt[b][:, :, :, 64:65], 1.0)

            def emit_qk_fast(b):
                """ch-outer, 2 passes of 2 query tiles: starts as chunks land.
                PSUM->SBUF evictions on the Scalar engine (DVE stays clear)."""
                for p_ in range(2):
                    q_ps = sps.tile([128, 1024], F32, tag="s", name="q_ps")
                    k_ps = sps.tile([128, 1024], F32, tag="s", name="k_ps")
                    for ch in range(N_CH):
                        for j in range(2):
                            t = 2 * p_ + j
                            rhs = x_sb[b][:, ch, t * 512:(t + 1) * 512]
                            nc.tensor.matmul(
                                q_ps[:, j * 512:(j + 1) * 512],
                                lhsT=wq_sb[:, ch, :], rhs=rhs,
                                start=(ch == 0), stop=(ch == N_CH - 1))
                        for j in range(2):
                            t = 2 * p_ + j
                            rhs = x_sb[b][:, ch, t * 512:(t + 1) * 512]
                            nc.tensor.matmul(
                                k_ps[:, j * 512:(j + 1) * 512],
                                lhsT=wk_sb[:, ch, :], rhs=rhs,
                                start=(ch == 0), stop=(ch == N_CH - 1))
                    nc.scalar.copy(Qt[b][:, p_ * 1024:(p_ + 1) * 1024], q_ps[:])
                    nc.scalar.copy(Kt[b][:, p_ * 1024:(p_ + 1) * 1024], k_ps[:])

            def qk_slice_steps(b, w_sb, dst, t):
                """ch-inner QK slice; psum borrowed from the (idle) av tag."""
                ps_ = avps.tile([128, 512], F32, tag="av0", name="qks_ps")
                for ch in range(N_CH):
                    nc.tensor.matmul(
                        ps_[:], lhsT=w_sb[:, ch, :],
                        rhs=x_sb[b][:, ch, t * 512:(t + 1) * 512],
                        start=(ch == 0), stop=(ch == N_CH - 1))
                    yield
                nc.scalar.copy(dst[:, t * 512:(t + 1) * 512], ps_[:])

            def v_chunk_steps(b, ck):
                v_ps = ops.tile([128, 512], F32, tag="o", name="v_ps")
                for ch in range(N_CH):
                    nc.tensor.matmul(
                        v_ps[:, 0:128],
                        lhsT=x_sb[b][:, ch, ck * 128:(ck + 1) * 128],
                        rhs=wv_sb[:, ch, :],
                        start=(ch == 0), stop=(ch == N_CH - 1))
                    yield
                nc.scalar.copy(
                    Vt[b][:, ck, :, 0:64],
                    v_ps[:, 0:128].rearrange("p (h e) -> p h e", e=64))

            a2a_in = [dram.tile([N_CORES, 128, S_SLICE], BF16, tag=f"a2ai{b}",
                                name=f"a2ai{b}") for b in range(B)]
            a2a_out = [dram.tile([N_CORES, 128, S_SLICE], BF16, tag=f"a2ao{b}",
                                 name=f"a2ao{b}") for b in range(B)]

            def emit_a2a(b):
                nc.gpsimd.collective_compute(
                    "AllToAll", mybir.AluOpType.bypass,
                    replica_groups=[list(range(N_CORES))],
                    ins=[a2a_in[b][:]], outs=[a2a_out[b][:]],
                )

            x28 = [x2p.tile([128, N_CH, S_SLICE], BF16, tag=f"x2_{b}",
                            name=f"x2_{b}") for b in range(B)]

            def emit_x2_loads(b):
                for src in range(N_CORES):
                    nc.sync.dma_start(x28[b][:, src, :], a2a_out[b][src])

            wot_sb = wp.tile([128, N_CH, D], BF16)
            bb_sb = wp.tile([128, D], F32)

            def outproj_steps(b, st, et):
                o_ps = ops.tile([128, 512], F32, tag="o", name="o_ps")
                for ch in range(N_CH):
                    nc.tensor.matmul(
                        o_ps[:],
                        lhsT=x28[b][:, ch, st * 128:(st + 1) * 128],
                        rhs=wot_sb[:, ch, et * 512:(et + 1) * 512],
                        start=(ch == 0), stop=(ch == N_CH - 1))
                    yield
                out_sb = outp.tile([128, 512], F32, tag="osb", name="out_sb")
                nc.vector.tensor_add(
                    out_sb[:], o_ps[:], bb_sb[:, et * 512:(et + 1) * 512])
                nc.sync.dma_start(
                    oc[b, st * 128:(st + 1) * 128, et * 512:(et + 1) * 512],
                    out_sb[:])
                yield

            def att_unit(b, t, fill=None, last=False):
                """Both heads for query tile t; AV software-pipelined by 2.
                Returns a deferred-normalize generator (consume in the next
                unit, or drain immediately for the last unit of a batch)."""
                qs = slice(t * 512, (t + 1) * 512)
                av = [avps.tile([65, 512], F32, tag=f"av{h}", name=f"av{h}")
                      for h in range(2)]
                p_hist = {}
                for c in range(N_KC + 2):
                    if fill is not None:
                        next(fill, None)
                    if c < N_KC:
                        j = c % 2
                        if j == 0:
                            p_hist[c // 2] = [
                                ptp.tile([128, 2, 512], BF16, tag=f"p{h}",
                                         name=f"p{h}") for h in range(2)]
                        s_ps = sps.tile([128, 1024], F32, tag="s", name="s_ps")
                        ks = slice(c * 128, (c + 1) * 128)
                        for h in range(2):
                            hp = slice(h * 64, (h + 1) * 64)
                            nc.tensor.matmul(
                                s_ps[:, h * 512:(h + 1) * 512],
                                lhsT=Kt[b][hp, ks], rhs=Qt[b][hp, qs],
                                start=True, stop=True)
                        pp = p_hist[c // 2]
                        nc.scalar.activation(
                            pp[0][:, j, :], s_ps[:, 0:512], EXP, scale=SCALE)
                        nc.vector.tensor_scalar(
                            pp[1][:, j, :].bitcast(I16),
                            s_ps[:, 512:1024], SCH_A, SCH_B,
                            AluOpType.mult, AluOpType.add)
                    ca = c - 2
                    if ca >= 0:
                        ja = ca % 2
                        pa = p_hist[ca // 2]
                        for h in range(2):
                            nc.tensor.matmul(
                                av[h][:], lhsT=Vt[b][:, ca, h, 0:65],
                                rhs=pa[h][:, ja, :],
                                start=(ca == 0), stop=(ca == N_KC - 1),
                                skip_group_check=True)
                        if ja == 1:
                            del p_hist[ca // 2]
                # evict av to SBUF (frees PSUM for the next unit quickly);
                # h0 eviction on Scalar, h1 on DVE
                av_sb = [avcp.tile([65, 512], F32, tag=f"avc{h}",
                                   name=f"avc{h}") for h in range(2)]
                nc.scalar.copy(av_sb[0][:], av[0][:])
                nc.scalar.copy(av_sb[1][:], av[1][:])
                # denominator rows -> DRAM bounce -> [64, 8] (64 DVE lanes)
                den64 = [normp.tile([64, 8], F32, tag=f"d64{h}",
                                    name=f"den64_{h}") for h in range(2)]
                for h in range(2):
                    den_d = dramsc.tile([512], F32, tag=f"dend{h}",
                                        name=f"den_d{h}")
                    nc.sync.dma_start(
                        den_d[:].rearrange("(a q) -> a q", a=1),
                        av_sb[h][64:65, :])
                    nc.sync.dma_start(
                        den64[h][:], den_d[:].rearrange("(p q) -> p q", p=64))

                def norm_steps():
                    for h in range(2):
                        rec64 = normp.tile([64, 8], F32, tag=f"r64{h}",
                                           name=f"rec64_{h}")
                        nc.vector.reciprocal(rec64[:], den64[h][:])
                        yield
                        rsc = dramsc.tile([512], F32, tag=f"rsc{h}",
                                          name=f"rsc{h}")
                        nc.sync.dma_start(
                            rsc[:].rearrange("(p q) -> p q", p=64), rec64[:])
                        bcast = normp.tile([64, 512], F32, tag=f"bc{h}",
                                           name=f"bcast{h}")
                        nc.sync.dma_start(
                            bcast[:],
                            rsc[:].rearrange("(a q) -> a q", a=1)
                            .broadcast_to([64, 512]))
                        yield
                        o_sb = normp.tile([64, 512], BF16, tag=f"ob{h}",
                                          name=f"o_sb{h}")
                        # deferred units use the idle GpSimd; the last unit of
                        # a batch is on the a2a critical path -> fast DVE
                        eng = nc.vector if last else nc.gpsimd
                        eng.tensor_mul(o_sb[:], av_sb[h][0:64, :], bcast[:])
                        for jj in range(2):
                            nc.sync.dma_start(
                                a2a_in[b][2 * t + jj, h * 64:(h + 1) * 64, :],
                                o_sb[:, jj * S_SLICE:(jj + 1) * S_SLICE])
                        yield

                return norm_steps()

            def chain(*gens):
                for g in gens:
                    yield from g

            # ================= pipeline =================
            emit_qk_fast(0)
            # x(1): query-tile-major so batch-1 prep unblocks progressively
            for t in range(N_QT):
                for ch in range(N_CH):
                    nc.sync.dma_start(
                        x_sb[1][:, ch, t * 512:(t + 1) * 512],
                        xtb[1, :, ch, t * 512:(t + 1) * 512])
            nc.sync.dma_start(wot_sb[:], wot[:])
            nc.sync.dma_start(bb_sb[:], bb[:])

            # Interleave the tiny V matmuls (N=128, ldw-bound) between the
            # N=512 QK-slice matmuls so their weight loads hide under the
            # long streams: pattern [QK, V, V] until a stream runs dry.
            def qk_gen():
                for t in range(N_QT):
                    yield from qk_slice_steps(1, wq_sb, Qt[1], t)
                    yield from qk_slice_steps(1, wk_sb, Kt[1], t)

            def v_gen():
                for ck in range(N_KC):
                    yield from v_chunk_steps(0, ck)
                for ck in range(N_KC):
                    yield from v_chunk_steps(1, ck)

            qg, vg = qk_gen(), v_gen()
            qk_live = v_live = True
            while qk_live or v_live:
                if v_live:
                    v_live = next(vg, StopIteration) is not StopIteration
                if v_live:
                    v_live = next(vg, StopIteration) is not StopIteration
                if qk_live:
                    qk_live = next(qg, StopIteration) is not StopIteration

            # batch-0 attention (normalize deferred into the next unit)
            nd = att_unit(0, 0)
            nd = att_unit(0, 1, fill=nd)
            nd = att_unit(0, 2, fill=nd)
            nd = att_unit(0, 3, fill=nd, last=True)
            for _ in nd:       # last batch-0 normalize: drain before a2a
                pass
            emit_a2a(0)
            emit_x2_loads(0)

            pieces = [(st, et) for st in range(S_SLICE // 128)
                      for et in range(D // 512)]

            nd = att_unit(1, 0)
            nd = att_unit(1, 1, fill=nd)
            nd = att_unit(1, 2, fill=chain(nd, outproj_steps(0, *pieces[0])))
            nd = att_unit(1, 3, fill=chain(nd, outproj_steps(0, *pieces[1])),
                          last=True)
            for _ in nd:
                pass
            emit_a2a(1)
            emit_x2_loads(1)
            # fill the a2a(1) window: remaining batch-0 pieces + warm-keeper
            # matmuls so the PE doesn't drop to the cold HAM clock
            for st, et in pieces[2:]:
                for _ in outproj_steps(0, st, et):
                    pass
            for _ in range(40):
                wps = ops.tile([128, 512], F32, tag="o", name="wps")
                nc.tensor.matmul(wps[:], lhsT=warm[:, 0:128], rhs=warm[:],
                                 start=True, stop=True)
            for st, et in pieces:
                for _ in outproj_steps(1, st, et):
                    pass

    nc.compile()
    return nc


def _prep_chunked(a_t):
    """[Din, E] (already transposed) -> [128, Din//128, E] chunk layout."""
    din, e = a_t.shape
    return np.ascontiguousarray(
        a_t.reshape(din // 128, 128, e).transpose(1, 0, 2)
    )


def kernel(x, w_qkv, w_out, b_out):
    global _compiled, last_results
    if _compiled is None:
        _compiled = _build()
    nc = _compiled

    x = np.asarray(x, dtype=np.float32)
    w_qkv = np.asarray(w_qkv, dtype=np.float32)
    w_out = np.asarray(w_out, dtype=np.float32)
    b_out = np.asarray(b_out, dtype=np.float32)

    xt_full = x.transpose(0, 2, 1)  # [B, D, S]
    xtb_prep = np.ascontiguousarray(
        xt_full.reshape(B, N_CH, 128, S).transpose(0, 2, 1, 3)
    ).astype(ml_dtypes.bfloat16)

    wot_prep = _prep_chunked(np.ascontiguousarray(w_out.T)).astype(
        ml_dtypes.bfloat16)
    bb_np = np.ascontiguousarray(np.broadcast_to(b_out, (128, D))).astype(
        np.float32)

    in_maps = []
    for c in range(N_CORES):
        hA, hB = HEADS_PER_CORE * c, HEADS_PER_CORE * c + 1
        rows = np.r_[hA * DH:(hA + 1) * DH, hB * DH:(hB + 1) * DH]
        wq = w_qkv[rows, :]
        wk = w_qkv[D + rows, :]
        wv = w_qkv[2 * D + rows, :]
        in_maps.append({
            "xtb": xtb_prep,
            "wqt": _prep_chunked(np.ascontiguousarray(wq.T)).astype(
                ml_dtypes.bfloat16),
            "wkt": _prep_chunked(np.ascontiguousarray(wk.T)).astype(
                ml_dtypes.bfloat16),
            "wvt": _prep_chunked(np.ascontiguousarray(wv.T)).astype(
                ml_dtypes.bfloat16),
            "wot": wot_prep,
            "bb": bb_np,
        })

    last_results = bass_utils.run_bass_kernel_spmd(
        nc, in_maps, core_ids=list(range(N_CORES))
    )
    out = np.concatenate(
        [last_results.results[c]["oc"] for c in range(N_CORES)], axis=1
    )
    return out
